# revision 33
# baseline (speedup 1.0000x reference)
"""Trainium2 Bass kernel for CDVectorQuantizer eval-mode forward.

Problem: z [32, 256, 4096] f32 (B, D, T), embedding [1024, 256] f32 (K, D).
For each token (b, t): idx = argmin_k ||z[b,:,t] - e_k||^2 ; out[b,:,t] = e_idx.

Math: argmin_k ||z-e_k||^2 == argmax_k (z.e_k - ||e_k||^2/2)  (||z||^2 const per token).

Sharding: data-parallel over batch B across 8 cores (4 batches/core), codebook
replicated. No collectives; host concatenates the per-core outputs.

Codebook-derived constants (weight preprocessing, host numpy, one-time per
call): eh = RNE-11 (f32r) rounding of e^T; el8 = e5m2(e^T - eh) residual;
bias = -||e_k||^2/2 replicated over 128 partitions; bf16 codebook for the
gather. Host f32r/e5m2 emulation was validated bit-exact vs HW casts.

Per-core kernel (SPMD on 8 cores), per 128-token tile:
  - scores [128,1024] on PE in 6 matmuls per tile: per 512-code chunk,
    2x f32r hi matmuls (zh.eh, f32r = RNE to 11 mantissa bits, 512 cyc each)
    + 1x fp8 DoubleRow lo matmul contracting all 256 dims at once:
    z8 (e4m3) x el8 (e5m2).  Net precision = exact e, 11-bit z: ~20 argmax
    flips / 131072 tokens, rel err ~1.7e-2 vs the 2e-2 gate, at 75% of the
    two-pass f32r PE cost.
  - bias is added inside a custom DVE op ARGMAX_BIAS_ANT (registered into
    dve_ops at import): argmax_k (scores[k] + bias[k]) in ONE pass reading
    scores directly from PSUM (no ScalarE staging copy) and bias from SBUF:
    body t = Src0+Src1; select(eq(t, scan(max,t)), Idx, 0), accum=max.
  - DVE converts the f32 index to u32 (tensor_scalar_min, also clamps),
    then GpSimd gathers codebook rows via indirect DMA.
  - [token,d]->[d,token] via PE transpose; ScalarE PSUM->SBUF copy; DMA out.
"""

import numpy as np
import ml_dtypes

import concourse.bacc as bacc
import concourse.bass as bass
import concourse.mybir as mybir
import concourse.tile as tile
from concourse.bass_utils import run_bass_kernel_spmd
from concourse.masks import make_identity

# Problem constants (hardcoded; kernel.py must be self-contained).
B, D, T = 32, 256, 4096
K = 1024
N_CORES = 8
BPC = B // N_CORES  # batches per core
P = 128
DCH = D // P        # 2 contraction chunks of 128
NCH = K // 512      # 2 code chunks of 512 (PSUM bank each)
TCHUNK = 1024       # tokens per z-load chunk
TT = TCHUNK // P    # token tiles per chunk (8)

F32 = mybir.dt.float32
F32R = mybir.dt.float32r
FP8E4 = mybir.dt.float8e4
FP8E5 = mybir.dt.float8e5
U8 = mybir.dt.uint8
U32 = mybir.dt.uint32
BF16 = mybir.dt.bfloat16
Alu = mybir.AluOpType
DR = mybir.MatmulPerfMode.DoubleRow


def register_argmax_bias_op():
    """Register the single-pass biased-argmax custom DVE op (idempotent).

    argmax_k (Src0[k] + Src1[k]) -> accum; reads Src0 from PSUM (scores)
    and Src1 from SBUF (bias broadcast)."""
    import concourse.dve_ops as dve_ops
    from concourse.dve_spec import Spec, Src0, Src1, Zero, AluOp, scan, eq, select, Idx, lower
    from concourse.dve_uop import DveOpSpec

    if "ARGMAX_BIAS_ANT" in dve_ops._SUB_OPCODE_FOR_NAME:
        return next(o for o in dve_ops.OPS if o.name == "ARGMAX_BIAS_ANT")

    def _ref(in0, in1, c0, c1, c2):
        s = in0 + in1
        r = np.maximum.accumulate(s, axis=-1)
        idxs = np.arange(in0.shape[-1], dtype=np.float32)
        body = np.where(s == r, idxs, 0.0).astype(np.float32)
        return body, body.max(axis=-1, keepdims=True)

    t = Src0 + Src1
    spec = Spec(
        body=select(eq(t, scan(AluOp.MAX, t)), Idx, Zero),
        accum=AluOp.MAX,
        reference=_ref,
    )
    shas = {}
    for ver in ("v3", "v4"):
        ds = DveOpSpec(
            name="ARGMAX_BIAS_ANT", opcode=0, uops=lower(spec, ver=ver), rd1_en=True
        )
        shas[ver] = ds.sha(ver)
    op = dve_ops.DveOp("ARGMAX_BIAS_ANT", spec, subdim=False, uops_sha=shas)
    dve_ops.OPS.append(op)
    dve_ops.CUSTOM_DVE_SPECS[op.name] = op.spec
    dve_ops._SUB_OPCODE_FOR_NAME[op.name] = (
        dve_ops._CUSTOM_DVE_ROW_BASE + len(dve_ops.OPS) - 1
    )
    return op


def build_vq_kernel():
    argmax_op = register_argmax_bias_op()
    nc = bacc.Bacc("TRN2", target_bir_lowering=False, debug=False)
    z = nc.dram_tensor("z", [BPC, D, T], F32R, kind="ExternalInput").ap()
    eh_in = nc.dram_tensor("eh_in", [DCH, P, K], F32R, kind="ExternalInput").ap()
    el8_in = nc.dram_tensor("el8_in", [P, DCH, K], FP8E5, kind="ExternalInput").ap()
    bias_in = nc.dram_tensor("bias_in", [P, K], F32, kind="ExternalInput").ap()
    emb_bf = nc.dram_tensor("emb_bf", [K, D], BF16, kind="ExternalInput").ap()
    out = nc.dram_tensor("out", [BPC, D, T], BF16, kind="ExternalOutput").ap()

    with tile.TileContext(nc) as tc:
        with tc.tile_pool(name="const", bufs=1) as const:
            identity = const.tile([P, P], F32)
            make_identity(nc, identity[:])
            identity_bf = const.tile([P, P], BF16, tag="id_bf")
            nc.vector.tensor_copy(out=identity_bf[:], in_=identity[:])
            eh = [const.tile([P, K], F32R, tag=f"eh{c}", name=f"eh{c}") for c in range(DCH)]
            el8 = const.tile([P, DCH, K], FP8E5, tag="el8")
            bias_bc = const.tile([P, K], F32, tag="bias_bc")
            for c in range(DCH):
                nc.sync.dma_start(out=eh[c][:], in_=eh_in[c])
            nc.sync.dma_start(out=el8[:], in_=el8_in)
            nc.sync.dma_start(out=bias_bc[:], in_=bias_in)

            # main-loop pools
            from contextlib import ExitStack
            _stack = ExitStack()
            zp = _stack.enter_context(tc.tile_pool(name="zpool", bufs=3))
            gp = _stack.enter_context(tc.tile_pool(name="gpool", bufs=4))
            sm = _stack.enter_context(tc.tile_pool(name="smpool", bufs=6))
            pss = _stack.enter_context(tc.tile_pool(name="ps_scores", bufs=3, space="PSUM"))
            pst = _stack.enter_context(tc.tile_pool(name="ps_tr", bufs=2, space="PSUM"))

            def prep_chunk_part(b, t0, c, state):
                """Issue DMA + casts for d-chunk c of chunk (b, t0)."""
                if c == 0:
                    state["z_hi"] = [None, None]
                    state["z8"] = zp.tile([P, TT, DCH, P], FP8E4, tag="z8", name="z8")
                z_raw = zp.tile([P, TCHUNK], F32R, tag=f"zr{c}", name=f"zr{c}")
                nc.sync.dma_start(
                    out=z_raw[:],
                    in_=z[b, c * P : (c + 1) * P, t0 : t0 + TCHUNK],
                )
                state["z_hi"][c] = z_raw
                nc.scalar.copy(
                    out=state["z8"][:, :, c, :],
                    in_=z_raw[:].bitcast(F32).rearrange("p (a b) -> p a b", b=P),
                )
                return state

            def prep_chunk(b, t0):
                state = {}
                prep_chunk_part(b, t0, 0, state)
                prep_chunk_part(b, t0, 1, state)
                return state

            prefetched = prep_chunk(0, 0)

            # ---------------- main loop ----------------
            pending = []
            FB = 4  # tiles per flush group

            def flush_group(items):
                # items: FB consecutive (gath, b, t0) entries
                fb, ft0 = items[0][1], items[0][2]
                trps = pst.tile([P, DCH, FB, P], BF16, tag="otrps", name="trps", bufs=2)
                for j, (gath, _, _) in enumerate(items):
                    for c in range(DCH):
                        nc.tensor.transpose(
                            out=trps[:, c, j, :],
                            in_=gath[:, c * P : (c + 1) * P],
                            identity=identity_bf[:],
                        )
                obuf = gp.tile([P, DCH, FB, P], BF16, tag="obuf", name="obuf", bufs=3)
                nc.scalar.copy(out=obuf[:], in_=trps[:])
                for c in range(DCH):
                    nc.sync.dma_start(
                        out=out[fb, c * P : (c + 1) * P, ft0 : ft0 + FB * P],
                        in_=obuf[:, c, :, :],
                    )

            chunks = [(b, t0) for b in range(BPC) for t0 in range(0, T, TCHUNK)]
            cur = prefetched
            for ci, (b, t0) in enumerate(chunks):
                    z_hi, z8 = cur["z_hi"], cur["z8"]
                    nxt = None
                    for tt in range(TT):
                        ts_ = slice(tt * P, (tt + 1) * P)
                        scores_ps = pss.tile([P, K], F32, tag="scores_ps")
                        # same-stationary pairs together, fp8 DR pass last --
                        # minimizes PE weight reloads and dtype-mode switches
                        for c in range(DCH):
                            for n in range(NCH):
                                ns = slice(n * 512, (n + 1) * 512)
                                nc.tensor.matmul(
                                    out=scores_ps[:, ns],
                                    lhsT=z_hi[c][:, ts_],
                                    rhs=eh[c][:, ns],
                                    start=(c == 0),
                                    stop=False,
                                )
                        for n in range(NCH):
                            ns = slice(n * 512, (n + 1) * 512)
                            nc.tensor.matmul(
                                out=scores_ps[:, ns],
                                lhsT=z8[:, tt, :, :],
                                rhs=el8[:, :, ns],
                                start=False,
                                stop=True,
                                perf_mode=DR,
                            )
                        # single-pass biased argmax on DVE, scores read from PSUM
                        junk = sm.tile([P, K], F32, tag="junk", bufs=2)
                        idxf = sm.tile([P, 1], F32, tag="idxf")
                        nc.vector._custom_dve(
                            argmax_op, out=junk[:], in0=scores_ps[:],
                            in1=bias_bc[:], accum_out=idxf[:],
                        )
                        # f32 -> u32 (+clamp) on DVE, then gather
                        idxu = sm.tile([P, 1], U32, tag="idxu")
                        nc.vector.tensor_scalar_min(
                            out=idxu[:], in0=idxf[:], scalar1=float(K - 1)
                        )
                        gath = gp.tile([P, D], BF16, tag="gath", bufs=32)
                        nc.gpsimd.indirect_dma_start(
                            out=gath[:],
                            out_offset=None,
                            in_=emb_bf[:],
                            in_offset=bass.IndirectOffsetOnAxis(ap=idxu[:], axis=0),
                        )
                        pending.append((gath, b, t0 + tt * P))
                        if len(pending) == 3 * FB:
                            flush_group(pending[0:FB])
                            del pending[0:FB]
                        # prefetch next chunk's z mid-chunk, split in two parts
                        # to avoid a single big ScalarE burst
                        if tt == 1 and ci + 1 < len(chunks):
                            nxt = prep_chunk_part(*chunks[ci + 1], 0, {})
                        if tt == 4 and ci + 1 < len(chunks):
                            nxt = prep_chunk_part(*chunks[ci + 1], 1, nxt)
                    cur = nxt
            while pending:
                flush_group(pending[0:FB])
                del pending[0:FB]
            _stack.close()
    nc.compile()
    return nc


_NC_CACHE = None


def _get_nc():
    global _NC_CACHE
    if _NC_CACHE is None:
        _NC_CACHE = build_vq_kernel()
    return _NC_CACHE


def _f32r_rne11(x: np.ndarray) -> np.ndarray:
    """RNE rounding of f32 to 11 explicit mantissa bits (matches TRN2 f32r;
    validated against HW casts)."""
    u = np.ascontiguousarray(x, dtype=np.float32).view(np.uint32)
    shift = 12  # 23 - 11
    lsb = (u >> shift) & 1
    u2 = (u + ((1 << (shift - 1)) - 1) + lsb) & (~np.uint32((1 << shift) - 1))
    return u2.view(np.float32)


def _prep_codebook(embedding: np.ndarray):
    """Host-side weight preprocessing of the codebook (one-time per call)."""
    embT = np.ascontiguousarray(embedding.T)            # [D, K]
    eh = _f32r_rne11(embT)                              # [D, K] f32r bits
    el = embT - eh                                      # exact f32 residual
    el8 = el.astype(ml_dtypes.float8_e5m2)
    eh_in = np.ascontiguousarray(eh.reshape(DCH, P, K))
    el8_in = np.ascontiguousarray(
        el8.reshape(DCH, P, K).transpose(1, 0, 2)       # [P, DCH, K]
    )
    bias = (-0.5 * (embedding.astype(np.float64) ** 2).sum(axis=1)).astype(np.float32)
    bias_in = np.ascontiguousarray(np.broadcast_to(bias[None, :], (P, K)))
    emb_bf = embedding.astype(ml_dtypes.bfloat16)
    return eh_in, el8_in, bias_in, emb_bf


def kernel(z: np.ndarray, embedding: np.ndarray, **run_kwargs) -> np.ndarray:
    z = np.ascontiguousarray(np.asarray(z, dtype=np.float32))
    embedding = np.ascontiguousarray(np.asarray(embedding, dtype=np.float32))
    assert z.shape == (B, D, T), z.shape
    assert embedding.shape == (K, D), embedding.shape

    eh_in, el8_in, bias_in, emb_bf = _prep_codebook(embedding)
    nc = _get_nc()
    in_maps = [
        {
            "z": z[i * BPC : (i + 1) * BPC],
            "eh_in": eh_in,
            "el8_in": el8_in,
            "bias_in": bias_in,
            "emb_bf": emb_bf,
        }
        for i in range(N_CORES)
    ]
    res = run_bass_kernel_spmd(nc, in_maps, core_ids=list(range(N_CORES)), **run_kwargs)
    out = np.concatenate(
        [np.asarray(r["out"]).astype(np.float32) for r in res.results], axis=0
    )
    if run_kwargs:
        kernel.last_results = res  # expose profile info to test harness
    return out


# revision 34
# speedup vs baseline: 1.1082x; 1.1082x over previous
"""Trainium2 Bass kernel for CDVectorQuantizer eval-mode forward.

Problem: z [32, 256, 4096] f32 (B, D, T), embedding [1024, 256] f32 (K, D).
For each token (b, t): idx = argmin_k ||z[b,:,t] - e_k||^2 ; out[b,:,t] = e_idx.

Math: argmin_k ||z-e_k||^2 == argmax_k (z.e_k - ||e_k||^2/2)  (||z||^2 const per token).

Sharding: data-parallel over batch B across 8 cores (4 batches/core), codebook
replicated. No collectives; host concatenates the per-core outputs.

Codebook-derived constants (weight preprocessing, host numpy, one-time per
call): eh = RNE-11 (f32r) rounding of e^T; el8 = e5m2(e^T - eh) residual;
bias = -||e_k||^2/2 replicated over 128 partitions; bf16 codebook for the
gather. Host f32r/e5m2 emulation was validated bit-exact vs HW casts.

Per-core kernel (SPMD on 8 cores), per 128-token tile:
  - scores [128,1024] on PE in 6 matmuls per tile: per 512-code chunk,
    2x f32r hi matmuls (zh.eh, f32r = RNE to 11 mantissa bits, 512 cyc each)
    + 1x fp8 DoubleRow lo matmul contracting all 256 dims at once:
    z8 (e4m3) x el8 (e5m2).  Net precision = exact e, 11-bit z: ~20 argmax
    flips / 131072 tokens, rel err ~1.7e-2 vs the 2e-2 gate, at 75% of the
    two-pass f32r PE cost.
  - bias is added inside a custom DVE op ARGMAX_BIAS_ANT (registered into
    dve_ops at import): argmax_k (scores[k] + bias[k]) in ONE pass reading
    scores directly from PSUM (no ScalarE staging copy) and bias from SBUF:
    body t = Src0+Src1; select(eq(t, scan(max,t)), Idx, 0), accum=max.
  - DVE converts the f32 index to u32 (tensor_scalar_min, also clamps),
    then GpSimd gathers codebook rows via indirect DMA.
  - [token,d]->[d,token] via PE transpose; ScalarE PSUM->SBUF copy; DMA out.
"""

import numpy as np
import ml_dtypes

import concourse.bacc as bacc
import concourse.bass as bass
import concourse.mybir as mybir
import concourse.tile as tile
from concourse.bass_utils import run_bass_kernel_spmd
from concourse.masks import make_identity

# Problem constants (hardcoded; kernel.py must be self-contained).
B, D, T = 32, 256, 4096
K = 1024
N_CORES = 8
BPC = B // N_CORES  # batches per core
P = 128
DCH = D // P        # 2 contraction chunks of 128
NCH = K // 512      # 2 code chunks of 512 (PSUM bank each)
TCHUNK = 1024       # tokens per z-load chunk
TT = TCHUNK // P    # token tiles per chunk (8)

F32 = mybir.dt.float32
F32R = mybir.dt.float32r
FP8E4 = mybir.dt.float8e4
FP8E5 = mybir.dt.float8e5
U8 = mybir.dt.uint8
U32 = mybir.dt.uint32
BF16 = mybir.dt.bfloat16
Alu = mybir.AluOpType
DR = mybir.MatmulPerfMode.DoubleRow


def register_argmax_bias_op():
    """Register the single-pass biased-argmax custom DVE op (idempotent).

    argmax_k (Src0[k] + Src1[k]) -> accum; reads Src0 from PSUM (scores)
    and Src1 from SBUF (bias broadcast)."""
    import concourse.dve_ops as dve_ops
    from concourse.dve_spec import Spec, Src0, Src1, Zero, AluOp, scan, eq, select, Idx, lower
    from concourse.dve_uop import DveOpSpec

    if "ARGMAX_BIAS_ANT" in dve_ops._SUB_OPCODE_FOR_NAME:
        return next(o for o in dve_ops.OPS if o.name == "ARGMAX_BIAS_ANT")

    def _ref(in0, in1, c0, c1, c2):
        s = in0 + in1
        r = np.maximum.accumulate(s, axis=-1)
        idxs = np.arange(in0.shape[-1], dtype=np.float32)
        body = np.where(s == r, idxs, 0.0).astype(np.float32)
        return body, body.max(axis=-1, keepdims=True)

    t = Src0 + Src1
    spec = Spec(
        body=select(eq(t, scan(AluOp.MAX, t)), Idx, Zero),
        accum=AluOp.MAX,
        reference=_ref,
    )
    shas = {}
    for ver in ("v3", "v4"):
        ds = DveOpSpec(
            name="ARGMAX_BIAS_ANT", opcode=0, uops=lower(spec, ver=ver), rd1_en=True
        )
        shas[ver] = ds.sha(ver)
    op = dve_ops.DveOp("ARGMAX_BIAS_ANT", spec, subdim=False, uops_sha=shas)
    dve_ops.OPS.append(op)
    dve_ops.CUSTOM_DVE_SPECS[op.name] = op.spec
    dve_ops._SUB_OPCODE_FOR_NAME[op.name] = (
        dve_ops._CUSTOM_DVE_ROW_BASE + len(dve_ops.OPS) - 1
    )
    return op


def build_vq_kernel():
    argmax_op = register_argmax_bias_op()
    nc = bacc.Bacc("TRN2", target_bir_lowering=False, debug=False)
    z = nc.dram_tensor("z", [BPC, D, T], F32, kind="ExternalInput").ap()
    eh_in = nc.dram_tensor("eh_in", [DCH, P, K], F32R, kind="ExternalInput").ap()
    el8_in = nc.dram_tensor("el8_in", [P, DCH, K], FP8E5, kind="ExternalInput").ap()
    bias_in = nc.dram_tensor("bias_in", [P, K], F32, kind="ExternalInput").ap()
    emb_bf = nc.dram_tensor("emb_bf", [K, D], BF16, kind="ExternalInput").ap()
    out = nc.dram_tensor("out", [BPC, D, T], BF16, kind="ExternalOutput").ap()

    with tile.TileContext(nc) as tc:
        with tc.tile_pool(name="const", bufs=1) as const:
            identity = const.tile([P, P], F32)
            make_identity(nc, identity[:])
            identity_bf = const.tile([P, P], BF16, tag="id_bf")
            nc.vector.tensor_copy(out=identity_bf[:], in_=identity[:])
            eh = [const.tile([P, K], F32R, tag=f"eh{c}", name=f"eh{c}") for c in range(DCH)]
            el8 = const.tile([P, DCH, K], FP8E5, tag="el8")
            bias_bc = const.tile([P, K], F32, tag="bias_bc")
            for c in range(DCH):
                nc.sync.dma_start(out=eh[c][:], in_=eh_in[c])
            nc.sync.dma_start(out=el8[:], in_=el8_in)
            nc.sync.dma_start(out=bias_bc[:], in_=bias_in)

            # main-loop pools
            from contextlib import ExitStack
            _stack = ExitStack()
            zp = _stack.enter_context(tc.tile_pool(name="zpool", bufs=3))
            gp = _stack.enter_context(tc.tile_pool(name="gpool", bufs=4))
            sm = _stack.enter_context(tc.tile_pool(name="smpool", bufs=6))
            pss = _stack.enter_context(tc.tile_pool(name="ps_scores", bufs=3, space="PSUM"))
            pst = _stack.enter_context(tc.tile_pool(name="ps_tr", bufs=2, space="PSUM"))

            def prep_chunk_part(b, t0, c, state):
                """Issue DMA + casts for d-chunk c of chunk (b, t0)."""
                if c == 0:
                    state["z_hi"] = [None, None]
                    state["z8"] = zp.tile([P, TT, DCH, P], FP8E4, tag="z8", name="z8")
                z_raw = zp.tile([P, TCHUNK], F32, tag=f"zr{c}", name=f"zr{c}")
                nc.sync.dma_start(
                    out=z_raw[:],
                    in_=z[b, c * P : (c + 1) * P, t0 : t0 + TCHUNK],
                )
                z_hi = zp.tile([P, TCHUNK], F32R, tag=f"zh{c}", name=f"zh{c}")
                nc.scalar.copy(out=z_hi[:], in_=z_raw[:])
                state["z_hi"][c] = z_hi
                nc.scalar.copy(
                    out=state["z8"][:, :, c, :],
                    in_=z_raw[:].rearrange("p (a b) -> p a b", b=P),
                )
                return state

            def prep_chunk(b, t0):
                state = {}
                prep_chunk_part(b, t0, 0, state)
                prep_chunk_part(b, t0, 1, state)
                return state

            prefetched = prep_chunk(0, 0)

            # ---------------- main loop ----------------
            pending = []
            FB = 4  # tiles per flush group

            def flush_group(items):
                # items: FB consecutive (gath, b, t0) entries
                fb, ft0 = items[0][1], items[0][2]
                trps = pst.tile([P, DCH, FB, P], BF16, tag="otrps", name="trps", bufs=2)
                for j, (gath, _, _) in enumerate(items):
                    for c in range(DCH):
                        nc.tensor.transpose(
                            out=trps[:, c, j, :],
                            in_=gath[:, c * P : (c + 1) * P],
                            identity=identity_bf[:],
                        )
                obuf = gp.tile([P, DCH, FB, P], BF16, tag="obuf", name="obuf", bufs=3)
                nc.scalar.copy(out=obuf[:], in_=trps[:])
                for c in range(DCH):
                    nc.sync.dma_start(
                        out=out[fb, c * P : (c + 1) * P, ft0 : ft0 + FB * P],
                        in_=obuf[:, c, :, :],
                    )

            chunks = [(b, t0) for b in range(BPC) for t0 in range(0, T, TCHUNK)]
            cur = prefetched
            for ci, (b, t0) in enumerate(chunks):
                    z_hi, z8 = cur["z_hi"], cur["z8"]
                    nxt = None
                    for tt in range(TT):
                        ts_ = slice(tt * P, (tt + 1) * P)
                        scores_ps = pss.tile([P, K], F32, tag="scores_ps")
                        # same-stationary pairs together, fp8 DR pass last --
                        # minimizes PE weight reloads and dtype-mode switches
                        for c in range(DCH):
                            for n in range(NCH):
                                ns = slice(n * 512, (n + 1) * 512)
                                nc.tensor.matmul(
                                    out=scores_ps[:, ns],
                                    lhsT=z_hi[c][:, ts_],
                                    rhs=eh[c][:, ns],
                                    start=(c == 0),
                                    stop=False,
                                )
                        for n in range(NCH):
                            ns = slice(n * 512, (n + 1) * 512)
                            nc.tensor.matmul(
                                out=scores_ps[:, ns],
                                lhsT=z8[:, tt, :, :],
                                rhs=el8[:, :, ns],
                                start=False,
                                stop=True,
                                perf_mode=DR,
                            )
                        # single-pass biased argmax on DVE, scores read from PSUM
                        junk = sm.tile([P, K], F32, tag="junk", bufs=2)
                        idxf = sm.tile([P, 1], F32, tag="idxf")
                        nc.vector._custom_dve(
                            argmax_op, out=junk[:], in0=scores_ps[:],
                            in1=bias_bc[:], accum_out=idxf[:],
                        )
                        # f32 -> u32 (+clamp) on DVE, then gather
                        idxu = sm.tile([P, 1], U32, tag="idxu")
                        nc.vector.tensor_scalar_min(
                            out=idxu[:], in0=idxf[:], scalar1=float(K - 1)
                        )
                        gath = gp.tile([P, D], BF16, tag="gath", bufs=32)
                        nc.gpsimd.indirect_dma_start(
                            out=gath[:],
                            out_offset=None,
                            in_=emb_bf[:],
                            in_offset=bass.IndirectOffsetOnAxis(ap=idxu[:], axis=0),
                        )
                        pending.append((gath, b, t0 + tt * P))
                        if len(pending) == 3 * FB:
                            flush_group(pending[0:FB])
                            del pending[0:FB]
                        # prefetch next chunk's z mid-chunk, split in two parts
                        # to avoid a single big ScalarE burst
                        if tt == 1 and ci + 1 < len(chunks):
                            nxt = prep_chunk_part(*chunks[ci + 1], 0, {})
                        if tt == 4 and ci + 1 < len(chunks):
                            nxt = prep_chunk_part(*chunks[ci + 1], 1, nxt)
                    cur = nxt
            while pending:
                flush_group(pending[0:FB])
                del pending[0:FB]
            _stack.close()
    nc.compile()
    return nc


_NC_CACHE = None


def _get_nc():
    global _NC_CACHE
    if _NC_CACHE is None:
        _NC_CACHE = build_vq_kernel()
    return _NC_CACHE


def _f32r_rne11(x: np.ndarray) -> np.ndarray:
    """RNE rounding of f32 to 11 explicit mantissa bits (matches TRN2 f32r;
    validated against HW casts)."""
    u = np.ascontiguousarray(x, dtype=np.float32).view(np.uint32)
    shift = 12  # 23 - 11
    lsb = (u >> shift) & 1
    u2 = (u + ((1 << (shift - 1)) - 1) + lsb) & (~np.uint32((1 << shift) - 1))
    return u2.view(np.float32)


def _prep_codebook(embedding: np.ndarray):
    """Host-side weight preprocessing of the codebook (one-time per call)."""
    embT = np.ascontiguousarray(embedding.T)            # [D, K]
    eh = _f32r_rne11(embT)                              # [D, K] f32r bits
    el = embT - eh                                      # exact f32 residual
    el8 = el.astype(ml_dtypes.float8_e5m2)
    eh_in = np.ascontiguousarray(eh.reshape(DCH, P, K))
    el8_in = np.ascontiguousarray(
        el8.reshape(DCH, P, K).transpose(1, 0, 2)       # [P, DCH, K]
    )
    bias = (-0.5 * (embedding.astype(np.float64) ** 2).sum(axis=1)).astype(np.float32)
    bias_in = np.ascontiguousarray(np.broadcast_to(bias[None, :], (P, K)))
    emb_bf = embedding.astype(ml_dtypes.bfloat16)
    return eh_in, el8_in, bias_in, emb_bf


def kernel(z: np.ndarray, embedding: np.ndarray, **run_kwargs) -> np.ndarray:
    z = np.ascontiguousarray(np.asarray(z, dtype=np.float32))
    embedding = np.ascontiguousarray(np.asarray(embedding, dtype=np.float32))
    assert z.shape == (B, D, T), z.shape
    assert embedding.shape == (K, D), embedding.shape

    eh_in, el8_in, bias_in, emb_bf = _prep_codebook(embedding)
    nc = _get_nc()
    in_maps = [
        {
            "z": z[i * BPC : (i + 1) * BPC],
            "eh_in": eh_in,
            "el8_in": el8_in,
            "bias_in": bias_in,
            "emb_bf": emb_bf,
        }
        for i in range(N_CORES)
    ]
    res = run_bass_kernel_spmd(nc, in_maps, core_ids=list(range(N_CORES)), **run_kwargs)
    out = np.concatenate(
        [np.asarray(r["out"]).astype(np.float32) for r in res.results], axis=0
    )
    if run_kwargs:
        kernel.last_results = res  # expose profile info to test harness
    return out
